# revision 6
# baseline (speedup 1.0000x reference)
"""Trainium2 Bass kernel for ANI-style per-species MLP (MoE routing).

Reference computation (dense form):
    h  = rep @ W1[e] + b1[e]          (no activation on layer-1 output)
    h  = relu(h @ Wh[e] + bh[e])
    en = h @ W2[e] + b2[e]
    out[b] = sum over atoms of en[b, a, species[b, a]]

Strategy: MoE dispatch on the host — gather atoms by species, split each
species' atom list evenly across the 8 NeuronCores, pad each per-species
group to a static capacity (multiple of 128).  Each core runs the same
Bass graph: for each species, grouped GEMMs (3-layer MLP) over that
species' atom columns, with the hidden dimension on SBUF partitions so
per-partition bias/ReLU epilogues fuse the PSUM->SBUF copy.  Compute is
bf16 (fp32 PSUM accumulation); measured rel err vs the fp32 reference is
~4e-3.  The per-atom energies are combined (scatter-add per molecule) on
the host.
"""

import numpy as np
import ml_dtypes

import concourse.bass as bass
from concourse import bacc
import concourse.mybir as mybir
from concourse import tile
from concourse.bass_utils import run_bass_kernel_spmd

B, A, D, E = 32, 1024, 384, 4
H1, H2 = 256, 192
NCORES = 8
DCH = D // 128          # 3 chunks of the descriptor dim
H1CH = H1 // 128        # 2 chunks of hidden-1
H2CH = 2                # hidden-2 padded 192 -> 256 = 2 chunks
MAX_N = 512             # moving free dim per matmul (one fp32 PSUM bank)
BF16 = ml_dtypes.bfloat16

_graph_cache = {}


def _chunks(c):
    """Split capacity c into matmul column tiles of at most MAX_N."""
    out = []
    n0 = 0
    while n0 < c:
        n = min(MAX_N, c - n0)
        out.append((n0, n))
        n0 += n
    return out


def _build_graph(caps):
    """One Bass graph, shared by all 8 cores (SPMD).

    caps: per-species atom-column capacity (each a multiple of 128).
    """
    offs = np.concatenate([[0], np.cumsum(caps)])
    ctot = int(offs[-1])
    f32, bf = mybir.dt.float32, mybir.dt.bfloat16
    Act = mybir.ActivationFunctionType

    nc = bacc.Bacc()
    xt_d = nc.declare_dram_parameter("xt", [128, DCH, ctot], bf, isOutput=False)
    w1_d = nc.declare_dram_parameter("w1", [128, E, DCH, H1CH, 128], bf, isOutput=False)
    b1_d = nc.declare_dram_parameter("b1", [128, E, H1CH, 1], f32, isOutput=False)
    wh_d = nc.declare_dram_parameter("wh", [128, E, H1CH, H2CH, 128], bf, isOutput=False)
    bh_d = nc.declare_dram_parameter("bh", [128, E, H2CH, 1], f32, isOutput=False)
    w2_d = nc.declare_dram_parameter("w2", [128, E, H2CH, 1], bf, isOutput=False)
    b2_d = nc.declare_dram_parameter("b2", [1, E, 1], f32, isOutput=False)
    out_d = nc.declare_dram_parameter("out", [1, ctot], f32, isOutput=True)

    with tile.TileContext(nc) as tc:
        with (
            tc.tile_pool(name="wp", bufs=1) as wp,
            tc.tile_pool(name="xp", bufs=4) as xp,
            tc.tile_pool(name="hp", bufs=2) as hp,
            tc.tile_pool(name="op", bufs=2) as op,
            tc.tile_pool(name="pp2", bufs=2, space="PSUM") as pp2,
            tc.tile_pool(name="pp1", bufs=1, space="PSUM") as pp1,
        ):
            w1_s = wp.tile([128, E, DCH, H1CH, 128], bf, tag="w1")
            nc.sync.dma_start(out=w1_s[:], in_=w1_d[:])
            b1_s = wp.tile([128, E, H1CH, 1], f32, tag="b1")
            nc.sync.dma_start(out=b1_s[:], in_=b1_d[:])
            wh_s = wp.tile([128, E, H1CH, H2CH, 128], bf, tag="wh")
            nc.sync.dma_start(out=wh_s[:], in_=wh_d[:])
            bh_s = wp.tile([128, E, H2CH, 1], f32, tag="bh")
            nc.sync.dma_start(out=bh_s[:], in_=bh_d[:])
            w2_s = wp.tile([128, E, H2CH, 1], bf, tag="w2")
            nc.sync.dma_start(out=w2_s[:], in_=w2_d[:])
            b2_s = wp.tile([1, E, 1], f32, tag="b2")
            nc.sync.dma_start(out=b2_s[:], in_=b2_d[:])

            for e in range(E):
                g0 = int(offs[e])
                for n0, n in _chunks(int(caps[e])):
                    x_t = xp.tile([128, DCH, n], bf, tag="x")
                    nc.sync.dma_start(
                        out=x_t[:], in_=xt_d[:, :, g0 + n0 : g0 + n0 + n]
                    )
                    h1_t = hp.tile([128, H1CH, n], bf, tag="h1")
                    h2_t = hp.tile([128, H2CH, n], bf, tag="h2")

                    # layer 1: h1.T = W1.T @ x.T + b1 (no activation)
                    for h in range(H1CH):
                        ps = pp2.tile([128, n], f32, tag=f"ph1_{h}")
                        for d in range(DCH):
                            nc.tensor.matmul(
                                ps[:],
                                lhsT=w1_s[:, e, d, h, :],
                                rhs=x_t[:, d, :],
                                start=(d == 0),
                                stop=(d == DCH - 1),
                            )
                        if h == 0:
                            nc.scalar.activation(
                                h1_t[:, h, :], ps[:], Act.Identity,
                                bias=b1_s[:, e, h, :],
                            )
                        else:
                            nc.vector.tensor_scalar_add(
                                h1_t[:, h, :], ps[:], b1_s[:, e, h, :]
                            )

                    # layer 2: h2.T = relu(Wh.T @ h1.T + bh)
                    for m in range(H2CH):
                        ps = pp1.tile([128, n], f32, tag=f"ph2_{m}")
                        for k in range(H1CH):
                            nc.tensor.matmul(
                                ps[:],
                                lhsT=wh_s[:, e, k, m, :],
                                rhs=h1_t[:, k, :],
                                start=(k == 0),
                                stop=(k == H1CH - 1),
                            )
                        if m == 0:
                            nc.scalar.activation(
                                h2_t[:, m, :], ps[:], Act.Relu,
                                bias=bh_s[:, e, m, :],
                            )
                        else:
                            nc.vector.tensor_scalar(
                                h2_t[:, m, :], ps[:], bh_s[:, e, m, :], 0.0,
                                mybir.AluOpType.add, mybir.AluOpType.max,
                            )

                    # layer 3: en = W2.T @ h2.T + b2
                    ps_e = pp2.tile([1, n], f32, tag="pen")
                    for k in range(H2CH):
                        nc.tensor.matmul(
                            ps_e[:],
                            lhsT=w2_s[:, e, k, :],
                            rhs=h2_t[:, k, :],
                            start=(k == 0),
                            stop=(k == H2CH - 1),
                        )
                    en_t = op.tile([1, n], f32, tag="en")
                    nc.scalar.activation(
                        en_t[:], ps_e[:], Act.Identity, bias=b2_s[:, e, :]
                    )
                    nc.sync.dma_start(
                        out=out_d[:, g0 + n0 : g0 + n0 + n], in_=en_t[:]
                    )
    return nc


def _pack_weights(W1, b1, Wh, bh, W2, b2):
    W1 = np.asarray(W1, np.float32)
    b1 = np.asarray(b1, np.float32)
    Wh = np.asarray(Wh, np.float32)
    bh = np.asarray(bh, np.float32)
    W2 = np.asarray(W2, np.float32)
    b2 = np.asarray(b2, np.float32)

    w1p = np.ascontiguousarray(
        W1.reshape(E, DCH, 128, H1CH, 128).transpose(2, 0, 1, 3, 4)
    ).astype(BF16)
    b1p = np.ascontiguousarray(
        b1.reshape(E, H1CH, 128).transpose(2, 0, 1)
    )[..., None]

    whp = np.zeros((E, H1, 128 * H2CH), np.float32)
    whp[:, :, :H2] = Wh
    whp = np.ascontiguousarray(
        whp.reshape(E, H1CH, 128, H2CH, 128).transpose(2, 0, 1, 3, 4)
    ).astype(BF16)
    bhp = np.zeros((E, 128 * H2CH), np.float32)
    bhp[:, :H2] = bh
    bhp = np.ascontiguousarray(
        bhp.reshape(E, H2CH, 128).transpose(2, 0, 1)
    )[..., None]
    w2p = np.zeros((E, 128 * H2CH), np.float32)
    w2p[:, :H2] = W2
    w2p = np.ascontiguousarray(
        w2p.reshape(E, H2CH, 128).transpose(2, 0, 1)
    ).astype(BF16)[..., None]
    b2p = np.ascontiguousarray(b2.reshape(1, E, 1))
    return {
        "w1": w1p, "b1": np.ascontiguousarray(b1p),
        "wh": whp, "bh": np.ascontiguousarray(bhp),
        "w2": w2p, "b2": b2p,
    }


def kernel(representation, species, W1, b1, Wh, bh, W2, b2):
    rep = np.ascontiguousarray(np.asarray(representation, np.float32)).reshape(
        B * A, D
    )
    spec = np.asarray(species).reshape(B * A)

    # --- dispatch: per-species atom lists, split evenly across cores ---
    idx_ce = [[None] * E for _ in range(NCORES)]
    for e in range(E):
        ide = np.nonzero(spec == e)[0]
        for c, part in enumerate(np.array_split(ide, NCORES)):
            idx_ce[c][e] = part
    caps = tuple(
        max(128, int(-(-max(len(idx_ce[c][e]) for c in range(NCORES)) // 128) * 128))
        for e in range(E)
    )
    offs = np.concatenate([[0], np.cumsum(caps)])
    ctot = int(offs[-1])

    wdict = _pack_weights(W1, b1, Wh, bh, W2, b2)
    rep_bf = rep.astype(BF16)

    in_maps = []
    for c in range(NCORES):
        xt = np.zeros((128, DCH, ctot), BF16)
        for e in range(E):
            ids = idx_ce[c][e]
            n = len(ids)
            if n:
                blk = rep_bf[ids].reshape(n, DCH, 128).transpose(2, 1, 0)
                xt[:, :, int(offs[e]) : int(offs[e]) + n] = blk
        in_maps.append({"xt": xt, **wdict})

    key = caps
    if key not in _graph_cache:
        nc = _build_graph(caps)
        nc.finalize()
        _graph_cache[key] = nc
    nc = _graph_cache[key]

    res = run_bass_kernel_spmd(nc, in_maps, core_ids=list(range(NCORES)))

    # --- combine: scatter-add per-atom energies into per-molecule sums ---
    out = np.zeros(B, np.float64)
    for c in range(NCORES):
        en = np.asarray(res.results[c]["out"], np.float64)[0]
        for e in range(E):
            ids = idx_ce[c][e]
            n = len(ids)
            if n:
                out += np.bincount(
                    ids // A,
                    weights=en[int(offs[e]) : int(offs[e]) + n],
                    minlength=B,
                )
    return out.astype(np.float32)


# revision 8
# speedup vs baseline: 1.2689x; 1.2689x over previous
"""Trainium2 Bass kernel for ANI-style per-species MLP (MoE routing).

Reference computation (dense form):
    h  = rep @ W1[e] + b1[e]          (no activation on layer-1 output)
    h  = relu(h @ Wh[e] + bh[e])
    en = h @ W2[e] + b2[e]
    out[b] = sum over atoms of en[b, a, species[b, a]]

Strategy: MoE dispatch on the host — gather atoms by species, split each
species' atom list evenly across the 8 NeuronCores, pad each per-species
group to a static capacity (multiple of 128).  Each core runs the same
Bass graph: for each species, grouped GEMMs (3-layer MLP) over that
species' atom columns, with the hidden dimension on SBUF partitions so
per-partition bias/ReLU epilogues fuse the PSUM->SBUF copy.  Compute is
bf16 (fp32 PSUM accumulation); measured rel err vs the fp32 reference is
~4e-3.  The per-atom energies are combined (scatter-add per molecule) on
the host.
"""

import numpy as np
import ml_dtypes

import concourse.bass as bass
from concourse import bacc
import concourse.mybir as mybir
from concourse import tile
from concourse.bass_utils import run_bass_kernel_spmd

B, A, D, E = 32, 1024, 384, 4
H1, H2 = 256, 192
NCORES = 8
DCH = D // 128          # 3 chunks of the descriptor dim
H1CH = H1 // 128        # 2 chunks of hidden-1
H2CH = 2                # hidden-2 padded 192 -> 256 = 2 chunks
MAX_N = 512             # moving free dim per matmul (one fp32 PSUM bank)
BF16 = ml_dtypes.bfloat16

_graph_cache = {}


def _chunks(c):
    """Split capacity c into matmul column tiles of at most MAX_N."""
    out = []
    n0 = 0
    while n0 < c:
        n = min(MAX_N, c - n0)
        out.append((n0, n))
        n0 += n
    return out


def _build_graph(caps):
    """One Bass graph, shared by all 8 cores (SPMD).

    caps: per-species atom-column capacity (each a multiple of 128).
    """
    offs = np.concatenate([[0], np.cumsum(caps)])
    ctot = int(offs[-1])
    f32, bf = mybir.dt.float32, mybir.dt.bfloat16
    Act = mybir.ActivationFunctionType

    nc = bacc.Bacc()
    xt_d = nc.declare_dram_parameter("xt", [128, DCH, ctot], bf, isOutput=False)
    w1_d = nc.declare_dram_parameter("w1", [128, E, DCH, H1CH, 128], bf, isOutput=False)
    b1_d = nc.declare_dram_parameter("b1", [128, E, H1CH, 1], f32, isOutput=False)
    wh_d = nc.declare_dram_parameter("wh", [128, E, H1CH, H2CH, 128], bf, isOutput=False)
    bh_d = nc.declare_dram_parameter("bh", [128, E, H2CH, 1], f32, isOutput=False)
    w2_d = nc.declare_dram_parameter("w2", [128, E, H2CH, 1], bf, isOutput=False)
    b2_d = nc.declare_dram_parameter("b2", [1, E, 1], f32, isOutput=False)
    out_d = nc.declare_dram_parameter("out", [1, ctot], f32, isOutput=True)

    with tile.TileContext(nc) as tc:
        with (
            tc.tile_pool(name="wp", bufs=1) as wp,
            tc.tile_pool(name="xp", bufs=6) as xp,
            tc.tile_pool(name="hp", bufs=2) as hp,
            tc.tile_pool(name="op", bufs=2) as op,
            tc.tile_pool(name="pp2", bufs=2, space="PSUM") as pp2,
            tc.tile_pool(name="pp1", bufs=1, space="PSUM") as pp1,
        ):
            w1_s = wp.tile([128, E, DCH, H1CH, 128], bf, tag="w1")
            nc.sync.dma_start(out=w1_s[:], in_=w1_d[:])
            b1_s = wp.tile([128, E, H1CH, 1], f32, tag="b1")
            nc.sync.dma_start(out=b1_s[:], in_=b1_d[:])
            wh_s = wp.tile([128, E, H1CH, H2CH, 128], bf, tag="wh")
            nc.sync.dma_start(out=wh_s[:], in_=wh_d[:])
            bh_s = wp.tile([128, E, H2CH, 1], f32, tag="bh")
            nc.sync.dma_start(out=bh_s[:], in_=bh_d[:])
            w2_s = wp.tile([128, E, H2CH, 1], bf, tag="w2")
            nc.sync.dma_start(out=w2_s[:], in_=w2_d[:])
            b2_s = wp.tile([1, E, 1], f32, tag="b2")
            nc.sync.dma_start(out=b2_s[:], in_=b2_d[:])

            for e in range(E):
                g0 = int(offs[e])
                for n0, n in _chunks(int(caps[e])):
                    x_t = xp.tile([128, DCH, n], bf, tag="x")
                    # scalar-engine HWDGE queue: keeps the x stream ordered and
                    # separate from the weight DMAs on the sync-engine queue
                    nc.scalar.dma_start(
                        out=x_t[:], in_=xt_d[:, :, g0 + n0 : g0 + n0 + n]
                    )
                    h1_t = hp.tile([128, H1CH, n], bf, tag="h1")
                    h2_t = hp.tile([128, H2CH, n], bf, tag="h2")

                    # layer 1: h1.T = W1.T @ x.T + b1 (no activation)
                    for h in range(H1CH):
                        ps = pp2.tile([128, n], f32, tag=f"ph1_{h}")
                        for d in range(DCH):
                            nc.tensor.matmul(
                                ps[:],
                                lhsT=w1_s[:, e, d, h, :],
                                rhs=x_t[:, d, :],
                                start=(d == 0),
                                stop=(d == DCH - 1),
                            )
                        if h == 0:
                            nc.scalar.activation(
                                h1_t[:, h, :], ps[:], Act.Identity,
                                bias=b1_s[:, e, h, :],
                            )
                        else:
                            nc.vector.tensor_scalar_add(
                                h1_t[:, h, :], ps[:], b1_s[:, e, h, :]
                            )

                    # layer 2: h2.T = relu(Wh.T @ h1.T + bh)
                    for m in range(H2CH):
                        ps = pp1.tile([128, n], f32, tag=f"ph2_{m}")
                        for k in range(H1CH):
                            nc.tensor.matmul(
                                ps[:],
                                lhsT=wh_s[:, e, k, m, :],
                                rhs=h1_t[:, k, :],
                                start=(k == 0),
                                stop=(k == H1CH - 1),
                            )
                        if m == 0:
                            nc.scalar.activation(
                                h2_t[:, m, :], ps[:], Act.Relu,
                                bias=bh_s[:, e, m, :],
                            )
                        else:
                            nc.vector.tensor_scalar(
                                h2_t[:, m, :], ps[:], bh_s[:, e, m, :], 0.0,
                                mybir.AluOpType.add, mybir.AluOpType.max,
                            )

                    # layer 3: en = W2.T @ h2.T + b2
                    ps_e = pp2.tile([1, n], f32, tag="pen")
                    for k in range(H2CH):
                        nc.tensor.matmul(
                            ps_e[:],
                            lhsT=w2_s[:, e, k, :],
                            rhs=h2_t[:, k, :],
                            start=(k == 0),
                            stop=(k == H2CH - 1),
                        )
                    en_t = op.tile([1, n], f32, tag="en")
                    nc.scalar.activation(
                        en_t[:], ps_e[:], Act.Identity, bias=b2_s[:, e, :]
                    )
                    nc.sync.dma_start(
                        out=out_d[:, g0 + n0 : g0 + n0 + n], in_=en_t[:]
                    )
    return nc


def _pack_weights(W1, b1, Wh, bh, W2, b2):
    W1 = np.asarray(W1, np.float32)
    b1 = np.asarray(b1, np.float32)
    Wh = np.asarray(Wh, np.float32)
    bh = np.asarray(bh, np.float32)
    W2 = np.asarray(W2, np.float32)
    b2 = np.asarray(b2, np.float32)

    w1p = np.ascontiguousarray(
        W1.reshape(E, DCH, 128, H1CH, 128).transpose(2, 0, 1, 3, 4)
    ).astype(BF16)
    b1p = np.ascontiguousarray(
        b1.reshape(E, H1CH, 128).transpose(2, 0, 1)
    )[..., None]

    whp = np.zeros((E, H1, 128 * H2CH), np.float32)
    whp[:, :, :H2] = Wh
    whp = np.ascontiguousarray(
        whp.reshape(E, H1CH, 128, H2CH, 128).transpose(2, 0, 1, 3, 4)
    ).astype(BF16)
    bhp = np.zeros((E, 128 * H2CH), np.float32)
    bhp[:, :H2] = bh
    bhp = np.ascontiguousarray(
        bhp.reshape(E, H2CH, 128).transpose(2, 0, 1)
    )[..., None]
    w2p = np.zeros((E, 128 * H2CH), np.float32)
    w2p[:, :H2] = W2
    w2p = np.ascontiguousarray(
        w2p.reshape(E, H2CH, 128).transpose(2, 0, 1)
    ).astype(BF16)[..., None]
    b2p = np.ascontiguousarray(b2.reshape(1, E, 1))
    return {
        "w1": w1p, "b1": np.ascontiguousarray(b1p),
        "wh": whp, "bh": np.ascontiguousarray(bhp),
        "w2": w2p, "b2": b2p,
    }


def kernel(representation, species, W1, b1, Wh, bh, W2, b2):
    rep = np.ascontiguousarray(np.asarray(representation, np.float32)).reshape(
        B * A, D
    )
    spec = np.asarray(species).reshape(B * A)

    # --- dispatch: per-species atom lists, split evenly across cores ---
    idx_ce = [[None] * E for _ in range(NCORES)]
    for e in range(E):
        ide = np.nonzero(spec == e)[0]
        for c, part in enumerate(np.array_split(ide, NCORES)):
            idx_ce[c][e] = part
    caps = tuple(
        max(128, int(-(-max(len(idx_ce[c][e]) for c in range(NCORES)) // 128) * 128))
        for e in range(E)
    )
    offs = np.concatenate([[0], np.cumsum(caps)])
    ctot = int(offs[-1])

    wdict = _pack_weights(W1, b1, Wh, bh, W2, b2)
    rep_bf = rep.astype(BF16)

    in_maps = []
    for c in range(NCORES):
        xt = np.zeros((128, DCH, ctot), BF16)
        for e in range(E):
            ids = idx_ce[c][e]
            n = len(ids)
            if n:
                blk = rep_bf[ids].reshape(n, DCH, 128).transpose(2, 1, 0)
                xt[:, :, int(offs[e]) : int(offs[e]) + n] = blk
        in_maps.append({"xt": xt, **wdict})

    key = caps
    if key not in _graph_cache:
        nc = _build_graph(caps)
        nc.finalize()
        _graph_cache[key] = nc
    nc = _graph_cache[key]

    res = run_bass_kernel_spmd(nc, in_maps, core_ids=list(range(NCORES)))

    # --- combine: scatter-add per-atom energies into per-molecule sums ---
    out = np.zeros(B, np.float64)
    for c in range(NCORES):
        en = np.asarray(res.results[c]["out"], np.float64)[0]
        for e in range(E):
            ids = idx_ce[c][e]
            n = len(ids)
            if n:
                out += np.bincount(
                    ids // A,
                    weights=en[int(offs[e]) : int(offs[e]) + n],
                    minlength=B,
                )
    return out.astype(np.float32)
